# revision 10
# baseline (speedup 1.0000x reference)
"""BernoulliRBF retrieval kernel for 8 trn2 NeuronCores.

Math: for each query n, over each reference set (pos/neg):
    score[n,m] = -(|xs_n - ys_m|^2 + wb) = 2 xs_n.ys_m - |ys_m|^2 - |xs_n|^2 - wb
    log_count[n] = LSE_m score[n,m]
Outputs: log_p_x = log_pos - logaddexp(log_pos, log_neg), log_weight_count.

Device work (per core; cores 0-3 hold pos shards, 4-7 neg shards of the
reference set, M-sharded):
    t[n,m] = 2 xs_n.ys_m - |ys_m|^2         (fp32r matmuls, bias via DVE add
                                             or a rank-2 ones-matmul)
    S[n,span] = sum_m exp(t[n,m] - C_n)      (ScalarE exp with accum)
with a per-query shift C_n chosen on the host from a reference subsample
(safe: C_n only needs to sit within ~±80 of the true row max; the sampled
max + margin is within ~±40 with huge margin). Host reduces partial sums
in float64: log_count = C_n + log(sum S) - |xs_n|^2 - wb.
"""
import os
import numpy as np
from contextlib import ExitStack

N, M, D = 2048, 32768, 256
NCORES = 8
CORES_PER_SET = 4
SHARD = M // CORES_PER_SET      # 8192 refs per core
NT = N // 128                   # 16 query tiles
GROUP = 1024                    # psum tile width (2 banks)
NG = SHARD // GROUP             # 8 groups per query tile
SPAN = 4                        # groups per ACT span (4096 columns)
NSPAN = NG // SPAN              # 2 spans per query tile
NSLOT = NT * NSPAN              # 32 accum slots per core
CN_MARGIN = 20.0
SAMPLE_STRIDE = 64              # 512-point subsample for C_n

LAST_EXEC_NS = None             # set when BASS_TRACE=1

_cache = {}


def _round_f32r(a):
    """Round fp32 -> fp32r (11-bit mantissa, RNE), keeping fp32 layout."""
    u = np.ascontiguousarray(a, dtype=np.float32).view(np.uint32)
    u = (u + np.uint32(0x7FF) + ((u >> np.uint32(12)) & np.uint32(1))) & np.uint32(
        0xFFFFF000
    )
    return u.view(np.float32)


def _build():
    import concourse.tile as tile
    from concourse import bacc, mybir

    F32, F32R = mybir.dt.float32, mybir.dt.float32r

    BF16 = mybir.dt.bfloat16
    nc = bacc.Bacc("TRN2", target_bir_lowering=False, debug=False)
    A = nc.dram_tensor("A", [2, 2, 128, N], BF16, kind="ExternalInput").ap()
    B = nc.dram_tensor("B", [2, 2, 128, SHARD], BF16, kind="ExternalInput").ap()
    BR = nc.dram_tensor("BR", [1, SHARD], F32, kind="ExternalInput").ap()
    CN = nc.dram_tensor("CN", [NT, 128], F32, kind="ExternalInput").ap()
    S = nc.dram_tensor("S", [128, NSLOT], F32, kind="ExternalOutput").ap()

    with tile.TileContext(nc) as tc:
        with ExitStack() as ctx:
            sing = ctx.enter_context(tc.tile_pool(name="sing", bufs=1))
            psums = ctx.enter_context(tc.tile_pool(name="psum", bufs=1, space="PSUM"))
            tts = ctx.enter_context(tc.tile_pool(name="tt", bufs=2))

            cn_sb = sing.tile([128, NT], F32)
            nc.sync.dma_start(out=cn_sb[:], in_=CN.rearrange("t p -> p t"))
            a_sb = sing.tile([128, 2, 2, N], BF16)
            for h in range(2):
                for d in range(2):
                    nc.sync.dma_start(out=a_sb[:, h, d, :], in_=A[h, d])
            b_sb = sing.tile([128, 2, 2, SHARD], BF16)
            w_sb = sing.tile([128, SHARD], F32)
            NBLK = 8
            for blk in range(NBLK):
                sl = slice(blk * (SHARD // NBLK), (blk + 1) * (SHARD // NBLK))
                for h in range(2):
                    for d in range(2):
                        nc.sync.dma_start(out=b_sb[:, h, d, sl], in_=B[h, d][:, sl])
                nc.sync.dma_start(
                    out=w_sb[:, sl], in_=BR[:, sl].to_broadcast([128, SHARD // NBLK])
                )
            s_sb = sing.tile([128, NSLOT], F32)
            dump = sing.tile([128, SPAN * GROUP], F32)
            psum_big = psums.tile([128, 4096], F32)

            SPAN_W = SPAN * GROUP          # 4096 columns per ACT span
            HALF_W = SPAN_W // 2           # 2048 columns per psum half
            hctr = 0
            for nt in range(NT):
                nsl = slice(nt * 128, (nt + 1) * 128)
                # weight sets: (a-part h, d-chunk) -> list of b-parts to stream
                wsets = [(0, 0, (0, 1)), (1, 0, (0,)), (0, 1, (0, 1)), (1, 1, (0,))]
                for span in range(NSPAN):
                    tt = tts.tile([128, SPAN_W], F32)
                    for hk in range(2):
                        p0 = (hctr % 2) * HALF_W
                        hctr += 1
                        m0 = span * SPAN_W + hk * HALF_W
                        for wi, (ah, d, bparts) in enumerate(wsets):
                            for c in range(HALF_W // 512):
                                for bi, bh in enumerate(bparts):
                                    nc.tensor.matmul(
                                        psum_big[:, p0 + c * 512:p0 + (c + 1) * 512],
                                        a_sb[:, ah, d, nsl],
                                        b_sb[:, bh, d, m0 + c * 512:m0 + (c + 1) * 512],
                                        start=(wi == 0 and bi == 0),
                                        stop=(wi == len(wsets) - 1),
                                    )
                        nc.vector.tensor_add(
                            tt[:, hk * HALF_W:(hk + 1) * HALF_W],
                            psum_big[:, p0:p0 + HALF_W],
                            w_sb[:, m0:m0 + HALF_W],
                        )
                    slot = nt * NSPAN + span
                    nc.scalar.activation(
                        out=dump[:],
                        in_=tt[:],
                        func=mybir.ActivationFunctionType.Exp,
                        bias=cn_sb[:, nt:nt + 1],
                        scale=1.0,
                        accum_out=s_sb[:, slot:slot + 1],
                    )
            nc.sync.dma_start(out=S[:], in_=s_sb[:])

    nc.compile()
    return nc


def _prep_set(x, data, scale):
    """Host-side prep for one reference set. Returns per-set tensors."""
    import ml_dtypes
    bf16 = np.dtype(ml_dtypes.bfloat16)
    xs = (x * scale[None, :]).astype(np.float32)          # match reference rounding
    ys = (data * scale[None, :]).astype(np.float32)
    a_t = np.ascontiguousarray((2.0 * xs).T).reshape(2, 128, N)
    a_hi = a_t.astype(bf16)
    a_lo = (a_t - a_hi.astype(np.float32)).astype(bf16)
    A = np.stack([a_hi, a_lo])                            # [h, d, 128, N] bf16
    b_t = np.ascontiguousarray(ys.T).reshape(2, 128, M)
    b_hi = b_t.astype(bf16)
    b_lo = (b_t - b_hi.astype(np.float32)).astype(bf16)
    BT = np.stack([b_hi, b_lo])                           # [h, d, 128, M] bf16
    br = -((ys.astype(np.float64) ** 2).sum(axis=1))      # [M], float64
    BR = br.astype(np.float32).reshape(1, M)
    # sampled per-query shift
    samp = ys[::SAMPLE_STRIDE]                            # [M/stride, D]
    t_s = 2.0 * (xs @ samp.T) + BR[0, ::SAMPLE_STRIDE][None, :]
    c_n = t_s.max(axis=1).astype(np.float64) + CN_MARGIN  # [N]
    c_n32 = c_n.astype(np.float32)                        # what the device sees
    CN = (-c_n32).reshape(NT, 128)
    xsq = (xs.astype(np.float64) ** 2).sum(axis=1)        # [N], float64
    return A, BT, BR, CN, c_n32.astype(np.float64), xsq


def kernel(x, data_pos, data_neg, scales_pos, scales_neg, weight_bias):
    global LAST_EXEC_NS
    from concourse.bass_utils import run_bass_kernel_spmd

    x = np.asarray(x, dtype=np.float32)
    data_pos = np.asarray(data_pos, dtype=np.float32)
    data_neg = np.asarray(data_neg, dtype=np.float32)
    scales_pos = np.asarray(scales_pos, dtype=np.float32)
    scales_neg = np.asarray(scales_neg, dtype=np.float32)
    weight_bias = np.asarray(weight_bias, dtype=np.float32)

    if "nc" not in _cache:
        _cache["nc"] = _build()
    nc = _cache["nc"]

    prep_p = _prep_set(x, data_pos, scales_pos)
    prep_n = _prep_set(x, data_neg, scales_neg)

    in_maps = []
    for core in range(NCORES):
        A_, B_, BR_, CN_, _, _ = prep_p if core < CORES_PER_SET else prep_n
        sh = core % CORES_PER_SET
        sl = slice(sh * SHARD, (sh + 1) * SHARD)
        in_maps.append(
            {
                "A": A_,
                "B": np.ascontiguousarray(B_[:, :, :, sl]),
                "BR": np.ascontiguousarray(BR_[:, sl]),
                "CN": CN_,
            }
        )

    trace = os.environ.get("BASS_TRACE", "") not in ("", "0")
    res = run_bass_kernel_spmd(nc, in_maps, list(range(NCORES)), trace=trace)
    LAST_EXEC_NS = res.exec_time_ns

    # host combine in float64
    def reduce_set(cores, c_shift, xsq, wb):
        tot = np.zeros(N)
        for core in cores:
            s = res.results[core]["S"].astype(np.float64)     # [128, NSLOT]
            per_n = s.reshape(128, NT, NSPAN).sum(axis=2)     # [p, nt]
            tot += per_n.T.reshape(N)
        return c_shift + np.log(tot) - xsq - float(wb)

    c_p, xsq_p = prep_p[4], prep_p[5]
    c_n, xsq_n = prep_n[4], prep_n[5]
    log_pos = reduce_set(range(CORES_PER_SET), c_p, xsq_p, weight_bias[0])
    log_neg = reduce_set(range(CORES_PER_SET, NCORES), c_n, xsq_n, weight_bias[1])
    log_weight = np.logaddexp(log_pos, log_neg)
    log_p_x = log_pos - log_weight
    return (log_p_x.astype(np.float32), log_weight.astype(np.float32))


# revision 11
# speedup vs baseline: 1.8939x; 1.8939x over previous
"""BernoulliRBF retrieval kernel for 8 trn2 NeuronCores.

Math: for each query n, over each reference set (pos/neg):
    score[n,m] = -(|xs_n - ys_m|^2 + wb) = 2 xs_n.ys_m - |ys_m|^2 - |xs_n|^2 - wb
    log_count[n] = LSE_m score[n,m]
Outputs: log_p_x = log_pos - logaddexp(log_pos, log_neg), log_weight_count.

Device work (per core; cores 0-3 hold pos shards, 4-7 neg shards of the
reference set, M-sharded):
    t[n,m] = 2 xs_n.ys_m - |ys_m|^2         (fp32r matmuls, bias via DVE add
                                             or a rank-2 ones-matmul)
    S[n,span] = sum_m exp(t[n,m] - C_n)      (ScalarE exp with accum)
with a per-query shift C_n chosen on the host from a reference subsample
(safe: C_n only needs to sit within ~±80 of the true row max; the sampled
max + margin is within ~±40 with huge margin). Host reduces partial sums
in float64: log_count = C_n + log(sum S) - |xs_n|^2 - wb.
"""
import os
import numpy as np
from contextlib import ExitStack

N, M, D = 2048, 32768, 256
NCORES = 8
CORES_PER_SET = 4
SHARD = M // CORES_PER_SET      # 8192 refs per core
NT = N // 128                   # 16 query tiles
GROUP = 1024                    # psum tile width (2 banks)
NG = SHARD // GROUP             # 8 groups per query tile
SPAN = 4                        # groups per ACT span (4096 columns)
NSPAN = NG // SPAN              # 2 spans per query tile
NSLOT = NT * NSPAN              # 32 accum slots per core
CN_MARGIN = 20.0
SAMPLE_STRIDE = 64              # 512-point subsample for C_n

LAST_EXEC_NS = None             # set when BASS_TRACE=1

_cache = {}


def _round_f32r(a):
    """Round fp32 -> fp32r (11-bit mantissa, RNE), keeping fp32 layout."""
    u = np.ascontiguousarray(a, dtype=np.float32).view(np.uint32)
    u = (u + np.uint32(0x7FF) + ((u >> np.uint32(12)) & np.uint32(1))) & np.uint32(
        0xFFFFF000
    )
    return u.view(np.float32)


def _build():
    import concourse.tile as tile
    from concourse import bacc, mybir

    F32, F32R = mybir.dt.float32, mybir.dt.float32r

    nc = bacc.Bacc("TRN2", target_bir_lowering=False, debug=False)
    A = nc.dram_tensor("A", [2, 128, N], F32R, kind="ExternalInput").ap()
    B = nc.dram_tensor("B", [2, 128, SHARD], F32R, kind="ExternalInput").ap()
    BR = nc.dram_tensor("BR", [1, SHARD], F32, kind="ExternalInput").ap()
    CN = nc.dram_tensor("CN", [NT, 128], F32, kind="ExternalInput").ap()
    S = nc.dram_tensor("S", [128, NSLOT], F32, kind="ExternalOutput").ap()

    with tile.TileContext(nc) as tc:
        with ExitStack() as ctx:
            sing = ctx.enter_context(tc.tile_pool(name="sing", bufs=1))
            psums = ctx.enter_context(tc.tile_pool(name="psum", bufs=1, space="PSUM"))
            tts = ctx.enter_context(tc.tile_pool(name="tt", bufs=2))

            cn_sb = sing.tile([128, NT], F32)
            nc.sync.dma_start(out=cn_sb[:], in_=CN.rearrange("t p -> p t"))
            a_sb = sing.tile([128, 2, N], F32R)
            for d in range(2):
                for nh in range(2):
                    sl = slice(nh * (N // 2), (nh + 1) * (N // 2))
                    nc.sync.dma_start(out=a_sb[:, d, sl], in_=A[d][:, sl])
            b_sb = sing.tile([128, 2, SHARD], F32R)
            w_sb = sing.tile([128, SHARD], F32)
            NBLK = 8
            for blk in range(NBLK):
                sl = slice(blk * (SHARD // NBLK), (blk + 1) * (SHARD // NBLK))
                for d in range(2):
                    nc.sync.dma_start(out=b_sb[:, d, sl], in_=B[d][:, sl])
                nc.sync.dma_start(
                    out=w_sb[:, sl], in_=BR[:, sl].to_broadcast([128, SHARD // NBLK])
                )
            s_sb = sing.tile([128, NSLOT], F32)
            dump = sing.tile([128, SPAN * GROUP], F32)
            psum_big = psums.tile([128, 4096], F32)

            SPAN_W = SPAN * GROUP          # 4096 columns per ACT span
            HALF_W = SPAN_W // 2           # 2048 columns per psum half
            hctr = 0
            for nt in range(NT):
                a_slices = [a_sb[:, d, nt * 128:(nt + 1) * 128] for d in range(2)]
                for span in range(NSPAN):
                    tt = tts.tile([128, SPAN_W], F32)
                    for hk in range(2):
                        p0 = (hctr % 2) * HALF_W
                        hctr += 1
                        m0 = span * SPAN_W + hk * HALF_W
                        for d in range(2):
                            for c in range(HALF_W // 512):
                                nc.tensor.matmul(
                                    psum_big[:, p0 + c * 512:p0 + (c + 1) * 512],
                                    a_slices[d],
                                    b_sb[:, d, m0 + c * 512:m0 + (c + 1) * 512],
                                    start=(d == 0),
                                    stop=(d == 1),
                                )
                        nc.vector.tensor_add(
                            tt[:, hk * HALF_W:(hk + 1) * HALF_W],
                            psum_big[:, p0:p0 + HALF_W],
                            w_sb[:, m0:m0 + HALF_W],
                        )
                    slot = nt * NSPAN + span
                    nc.scalar.activation(
                        out=dump[:],
                        in_=tt[:],
                        func=mybir.ActivationFunctionType.Exp,
                        bias=cn_sb[:, nt:nt + 1],
                        scale=1.0,
                        accum_out=s_sb[:, slot:slot + 1],
                    )
            nc.sync.dma_start(out=S[:], in_=s_sb[:])

    nc.compile()
    return nc


def _prep_set(x, data, scale):
    """Host-side prep for one reference set. Returns per-set tensors."""
    xs = (x * scale[None, :]).astype(np.float32)          # match reference rounding
    ys = (data * scale[None, :]).astype(np.float32)
    A = _round_f32r(np.ascontiguousarray((2.0 * xs).T).reshape(2, 128, N))
    BT = _round_f32r(np.ascontiguousarray(ys.T).reshape(2, 128, M))
    br = -((ys.astype(np.float64) ** 2).sum(axis=1))      # [M], float64
    BR = br.astype(np.float32).reshape(1, M)
    # sampled per-query shift
    samp = ys[::SAMPLE_STRIDE]                            # [M/stride, D]
    t_s = 2.0 * (xs @ samp.T) + BR[0, ::SAMPLE_STRIDE][None, :]
    c_n = t_s.max(axis=1).astype(np.float64) + CN_MARGIN  # [N]
    c_n32 = c_n.astype(np.float32)                        # what the device sees
    CN = (-c_n32).reshape(NT, 128)
    xsq = (xs.astype(np.float64) ** 2).sum(axis=1)        # [N], float64
    return A, BT, BR, CN, c_n32.astype(np.float64), xsq


def kernel(x, data_pos, data_neg, scales_pos, scales_neg, weight_bias):
    global LAST_EXEC_NS
    from concourse.bass_utils import run_bass_kernel_spmd

    x = np.asarray(x, dtype=np.float32)
    data_pos = np.asarray(data_pos, dtype=np.float32)
    data_neg = np.asarray(data_neg, dtype=np.float32)
    scales_pos = np.asarray(scales_pos, dtype=np.float32)
    scales_neg = np.asarray(scales_neg, dtype=np.float32)
    weight_bias = np.asarray(weight_bias, dtype=np.float32)

    if "nc" not in _cache:
        _cache["nc"] = _build()
    nc = _cache["nc"]

    prep_p = _prep_set(x, data_pos, scales_pos)
    prep_n = _prep_set(x, data_neg, scales_neg)

    in_maps = []
    for core in range(NCORES):
        A_, B_, BR_, CN_, _, _ = prep_p if core < CORES_PER_SET else prep_n
        sh = core % CORES_PER_SET
        sl = slice(sh * SHARD, (sh + 1) * SHARD)
        in_maps.append(
            {
                "A": A_,
                "B": np.ascontiguousarray(B_[:, :, sl]),
                "BR": np.ascontiguousarray(BR_[:, sl]),
                "CN": CN_,
            }
        )

    trace = os.environ.get("BASS_TRACE", "") not in ("", "0")
    res = run_bass_kernel_spmd(nc, in_maps, list(range(NCORES)), trace=trace)
    LAST_EXEC_NS = res.exec_time_ns

    # host combine in float64
    def reduce_set(cores, c_shift, xsq, wb):
        tot = np.zeros(N)
        for core in cores:
            s = res.results[core]["S"].astype(np.float64)     # [128, NSLOT]
            per_n = s.reshape(128, NT, NSPAN).sum(axis=2)     # [p, nt]
            tot += per_n.T.reshape(N)
        return c_shift + np.log(tot) - xsq - float(wb)

    c_p, xsq_p = prep_p[4], prep_p[5]
    c_n, xsq_n = prep_n[4], prep_n[5]
    log_pos = reduce_set(range(CORES_PER_SET), c_p, xsq_p, weight_bias[0])
    log_neg = reduce_set(range(CORES_PER_SET, NCORES), c_n, xsq_n, weight_bias[1])
    log_weight = np.logaddexp(log_pos, log_neg)
    log_p_x = log_pos - log_weight
    return (log_p_x.astype(np.float32), log_weight.astype(np.float32))
